# revision 1
# baseline (speedup 1.0000x reference)
"""Trainium2 Bass kernel: Mistral quantized MLP (SwiGLU with int8-valued int32
weights, per-output-channel scales).

  gate = (x @ dequant(gate_wq).T), up = (x @ dequant(up_wq).T)
  h = silu(gate) * up
  out = h @ dequant(down_wq).T

Strategy (8 NeuronCores, tensor-parallel on the intermediate dim I):
  - Core c owns rows [c*I/8, (c+1)*I/8) of gate/up and the matching columns of
    down. Each core computes a full [H, T] partial of the down projection;
    the host sums the 8 partials (the "all-reduce"), applies down_scale, and
    transposes back to [B, S, H].
  - Weights are int8-valued, hence EXACTLY representable in bf16. Activations
    are split hi/lo into two bf16 tensors (x = hi + lo, likewise the SwiGLU
    output h), so every matmul runs at bf16 speed with fp32 PSUM accumulation
    and ~1e-5 overall relative error.
  - Device layout keeps features on partitions and tokens on the free dim:
    x is pre-transposed/tiled on the host to [mega, ki, ko, t]; weights to
    [o_tile, ki, ko, o] so each DMA is contiguous and each matmul is
    lhsT=[128 k, 128 o] stationary x rhs=[128 k, 512 t] moving.
"""

import os

import ml_dtypes
import numpy as np

_BF16 = ml_dtypes.bfloat16

# Problem dims (hardcoded per the task contract).
B, S, H, I = 2, 2048, 4096, 14336
NCORES = 8
I_LOC = I // NCORES  # 1792
T = B * S  # 4096
T_MEGA = 512  # tokens per resident x block (and per-matmul free dim)

_nc_cache = {}


def _build_module(t_mega, n_mega, ko_g, ot_g, ot_d):
    """Build + compile the (SPMD, identical on all cores) Bass module.

    ko_g: contraction tiles for gate/up (H/128)
    ot_g: output tiles per core for gate/up (I_loc/128); also the down
          contraction tile count
    ot_d: output tiles for down (H/128)
    """
    import concourse.tile as tile
    from concourse import bacc, mybir

    f32 = mybir.dt.float32
    bf16 = mybir.dt.bfloat16
    silu = mybir.ActivationFunctionType.Silu
    mult = mybir.AluOpType.mult
    ko_d = ot_g

    nc = bacc.Bacc(
        "TRN2",
        target_bir_lowering=False,
        debug=False,
        enable_asserts=False,
        num_devices=NCORES,
    )

    xh_d = nc.dram_tensor(
        "x_hi", [n_mega, 128, ko_g, t_mega], bf16, kind="ExternalInput"
    ).ap()
    xl_d = nc.dram_tensor(
        "x_lo", [n_mega, 128, ko_g, t_mega], bf16, kind="ExternalInput"
    ).ap()
    gw_d = nc.dram_tensor(
        "gate_w", [ot_g, 128, ko_g, 128], bf16, kind="ExternalInput"
    ).ap()
    uw_d = nc.dram_tensor(
        "up_w", [ot_g, 128, ko_g, 128], bf16, kind="ExternalInput"
    ).ap()
    dw_d = nc.dram_tensor(
        "down_w", [ot_d, 128, ko_d, 128], bf16, kind="ExternalInput"
    ).ap()
    gs_d = nc.dram_tensor("gate_s", [128, ot_g], f32, kind="ExternalInput").ap()
    us_d = nc.dram_tensor("up_s", [128, ot_g], f32, kind="ExternalInput").ap()
    out_d = nc.dram_tensor(
        "out", [ot_d * 128, n_mega * t_mega], f32, kind="ExternalOutput"
    ).ap()

    with tile.TileContext(nc) as tc:
        with (
            tc.tile_pool(name="px", bufs=1) as px,
            tc.tile_pool(name="pw", bufs=2) as pw,
            tc.tile_pool(name="pdw", bufs=2) as pdw,
            tc.tile_pool(name="ph", bufs=1) as ph,
            tc.tile_pool(name="pe", bufs=2) as pe,
            tc.tile_pool(name="po", bufs=3) as po,
            tc.tile_pool(name="pscale", bufs=1) as pscale,
            tc.tile_pool(name="pp", bufs=8, space="PSUM") as pp,
        ):
            gs_t = pscale.tile([128, ot_g], f32, name="gs_t")
            nc.sync.dma_start(out=gs_t[:], in_=gs_d[:])
            us_t = pscale.tile([128, ot_g], f32, name="us_t")
            nc.sync.dma_start(out=us_t[:], in_=us_d[:])

            for m in range(n_mega):
                xh = px.tile([128, ko_g, t_mega], bf16, tag="xh", name="xh")
                nc.sync.dma_start(out=xh[:], in_=xh_d[m])
                xl = px.tile([128, ko_g, t_mega], bf16, tag="xl", name="xl")
                nc.sync.dma_start(out=xl[:], in_=xl_d[m])
                hh = ph.tile([128, ko_d, t_mega], bf16, tag="hh", name="hh")
                hl = ph.tile([128, ko_d, t_mega], bf16, tag="hl", name="hl")

                # ---- gate/up matmuls + SwiGLU epilogue ----
                for ot in range(ot_g):
                    gw = pw.tile([128, ko_g, 128], bf16, tag="gw", name="gw")
                    nc.sync.dma_start(out=gw[:], in_=gw_d[ot])
                    uw = pw.tile([128, ko_g, 128], bf16, tag="uw", name="uw")
                    nc.sync.dma_start(out=uw[:], in_=uw_d[ot])

                    psg = pp.tile([128, t_mega], f32, tag="ps", name="psg")
                    for k in range(ko_g):
                        nc.tensor.matmul(
                            psg[:], gw[:, k, :], xh[:, k, :], start=(k == 0), stop=False
                        )
                        nc.tensor.matmul(
                            psg[:], gw[:, k, :], xl[:, k, :],
                            start=False, stop=(k == ko_g - 1),
                        )
                    psu = pp.tile([128, t_mega], f32, tag="ps", name="psu")
                    for k in range(ko_g):
                        nc.tensor.matmul(
                            psu[:], uw[:, k, :], xh[:, k, :], start=(k == 0), stop=False
                        )
                        nc.tensor.matmul(
                            psu[:], uw[:, k, :], xl[:, k, :],
                            start=False, stop=(k == ko_g - 1),
                        )

                    gact = pe.tile([128, t_mega], f32, tag="gact", name="gact")
                    nc.scalar.activation(
                        gact[:], psg[:], silu, scale=gs_t[:, ot : ot + 1]
                    )
                    # prod = (up_psum * up_scale) * silu(gate * gate_scale)
                    prod = pe.tile([128, t_mega], f32, tag="prod", name="prod")
                    nc.vector.scalar_tensor_tensor(
                        prod[:], psu[:], us_t[:, ot : ot + 1], gact[:], mult, mult
                    )
                    nc.vector.tensor_copy(out=hh[:, ot, :], in_=prod[:])
                    nc.vector.tensor_sub(hl[:, ot, :], prod[:], hh[:, ot, :])

                # ---- down matmuls (partial sums; scaled on host) ----
                for o2 in range(ot_d):
                    dw = pdw.tile([128, ko_d, 128], bf16, tag="dw", name="dw")
                    nc.sync.dma_start(out=dw[:], in_=dw_d[o2])
                    pso = pp.tile([128, t_mega], f32, tag="ps", name="pso")
                    for k in range(ko_d):
                        nc.tensor.matmul(
                            pso[:], dw[:, k, :], hh[:, k, :], start=(k == 0), stop=False
                        )
                        nc.tensor.matmul(
                            pso[:], dw[:, k, :], hl[:, k, :],
                            start=False, stop=(k == ko_d - 1),
                        )
                    ob = po.tile([128, t_mega], f32, tag="ob", name="ob")
                    nc.scalar.copy(ob[:], pso[:])
                    nc.sync.dma_start(
                        out=out_d[
                            o2 * 128 : (o2 + 1) * 128,
                            m * t_mega : (m + 1) * t_mega,
                        ],
                        in_=ob[:],
                    )

    nc.compile()
    return nc


def _get_module(key, t_mega, n_mega, ko_g, ot_g, ot_d):
    if key not in _nc_cache:
        _nc_cache[key] = _build_module(t_mega, n_mega, ko_g, ot_g, ot_d)
    return _nc_cache[key]


def _prep_x(x, t_mega, n_mega, ko_g):
    """[T, H] f32 -> hi/lo bf16 tiled [mega, ki, ko, t]."""
    t_total = n_mega * t_mega
    xf = np.ascontiguousarray(x.reshape(t_total, ko_g * 128), dtype=np.float32)
    xr = xf.reshape(n_mega, t_mega, ko_g, 128).transpose(0, 3, 2, 1)
    x_hi = np.ascontiguousarray(xr).astype(_BF16)
    x_lo = (xr - x_hi.astype(np.float32)).astype(_BF16)
    return x_hi, x_lo


def _prep_w(w_bf, ot, ko):
    """[ot*128 (o), ko*128 (k)] bf16 -> [ot, ki, ko, o] contiguous."""
    return np.ascontiguousarray(
        w_bf.reshape(ot, 128, ko, 128).transpose(0, 3, 2, 1)
    )


def _prep_scale(s, ot):
    return np.ascontiguousarray(s.reshape(ot, 128).T, dtype=np.float32)


def _run_spmd(nc, in_maps, trace):
    from concourse.bass_utils import run_bass_kernel_spmd

    return run_bass_kernel_spmd(
        nc, in_maps, core_ids=list(range(len(in_maps))), trace=trace
    )


def kernel(x, gate_wq, gate_scale, up_wq, up_scale, down_wq, down_scale):
    n_mega = T // T_MEGA
    ko_g = H // 128
    ot_g = I_LOC // 128
    ot_d = H // 128

    nc = _get_module("full", T_MEGA, n_mega, ko_g, ot_g, ot_d)

    x_hi, x_lo = _prep_x(np.asarray(x), T_MEGA, n_mega, ko_g)
    gate_bf = np.asarray(gate_wq).astype(_BF16)  # int8-valued -> exact
    up_bf = np.asarray(up_wq).astype(_BF16)
    down_bf = np.asarray(down_wq).astype(_BF16)
    gate_scale = np.asarray(gate_scale, dtype=np.float32)
    up_scale = np.asarray(up_scale, dtype=np.float32)
    down_scale = np.asarray(down_scale, dtype=np.float32)

    in_maps = []
    for c in range(NCORES):
        sl = slice(c * I_LOC, (c + 1) * I_LOC)
        in_maps.append(
            {
                "x_hi": x_hi,
                "x_lo": x_lo,
                "gate_w": _prep_w(gate_bf[sl], ot_g, ko_g),
                "up_w": _prep_w(up_bf[sl], ot_g, ko_g),
                "down_w": _prep_w(down_bf[:, sl], ot_d, ot_g),
                "gate_s": _prep_scale(gate_scale[sl], ot_g),
                "up_s": _prep_scale(up_scale[sl], ot_g),
            }
        )

    trace = bool(int(os.environ.get("TRNMLP_TRACE", "0")))
    res = _run_spmd(nc, in_maps, trace)
    if trace:
        kernel.last_results = res

    acc = res.results[0]["out"].astype(np.float32, copy=True)
    for r in res.results[1:]:
        acc += r["out"]
    acc *= down_scale[:, None]
    return np.ascontiguousarray(acc.T).reshape(B, S, H).astype(np.float32)


kernel.last_results = None


# revision 2
# speedup vs baseline: 1.7903x; 1.7903x over previous
"""Trainium2 Bass kernel: Mistral quantized MLP (SwiGLU with int8-valued int32
weights, per-output-channel scales).

  gate = (x @ dequant(gate_wq).T), up = (x @ dequant(up_wq).T)
  h = silu(gate) * up
  out = h @ dequant(down_wq).T

Strategy (8 NeuronCores, tensor-parallel on the intermediate dim I):
  - Core c owns rows [c*I/8, (c+1)*I/8) of gate/up and the matching columns of
    down. Each core computes a full [H, T] partial of the down projection;
    the host sums the 8 partials (the "all-reduce"), applies down_scale, and
    transposes back to [B, S, H].
  - Weights are int8-valued, hence EXACTLY representable in fp16/bf16.
  - Two precision modes (TRNMLP_MODE env var):
      "fp16"       (default): activations in fp16 (11-bit mantissa), weights
                   exact fp16. One matmul pass. ~3.6e-4 L2 rel err.
      "split_bf16": activations split hi/lo into two bf16 tensors, weights
                   exact bf16. Two matmul passes. ~4.3e-6 L2 rel err, 2x PE.
  - Device layout keeps features on partitions, tokens on the free dim:
    x is pre-transposed/tiled on the host to [mega, ki, ko, t]; weights to
    [o_tile, ki, ko, o] so each DMA is contiguous and each matmul is
    lhsT=[128 k, 128 o] stationary x rhs=[128 k, 512 t] moving, fp32 PSUM.
"""

import os

import ml_dtypes
import numpy as np

_BF16 = ml_dtypes.bfloat16

# Problem dims (hardcoded per the task contract).
B, S, H, I = 2, 2048, 4096, 14336
NCORES = 8
I_LOC = I // NCORES  # 1792
T = B * S  # 4096
T_MEGA = 512  # tokens per resident x block (and per-matmul free dim)

MODE = os.environ.get("TRNMLP_MODE", "fp16")  # "fp16" | "split_bf16"

_nc_cache = {}


def _build_module(mode, t_mega, n_mega, ko_g, ot_g, ot_d):
    """Build + compile the (SPMD, identical on all cores) Bass module.

    ko_g: contraction tiles for gate/up (H/128)
    ot_g: output tiles per core for gate/up (I_loc/128); also the down
          contraction tile count
    ot_d: output tiles for down (H/128)
    """
    import concourse.tile as tile
    from concourse import bacc, mybir

    f32 = mybir.dt.float32
    act_dt = mybir.dt.float16 if mode == "fp16" else mybir.dt.bfloat16
    silu = mybir.ActivationFunctionType.Silu
    mult = mybir.AluOpType.mult
    ko_d = ot_g
    split = mode == "split_bf16"

    nc = bacc.Bacc(
        "TRN2",
        target_bir_lowering=False,
        debug=False,
        enable_asserts=False,
        num_devices=NCORES,
    )

    xh_d = nc.dram_tensor(
        "x_hi", [n_mega, 128, ko_g, t_mega], act_dt, kind="ExternalInput"
    ).ap()
    if split:
        xl_d = nc.dram_tensor(
            "x_lo", [n_mega, 128, ko_g, t_mega], act_dt, kind="ExternalInput"
        ).ap()
    gw_d = nc.dram_tensor(
        "gate_w", [ot_g, 128, ko_g, 128], act_dt, kind="ExternalInput"
    ).ap()
    uw_d = nc.dram_tensor(
        "up_w", [ot_g, 128, ko_g, 128], act_dt, kind="ExternalInput"
    ).ap()
    dw_d = nc.dram_tensor(
        "down_w", [ot_d, 128, ko_d, 128], act_dt, kind="ExternalInput"
    ).ap()
    gs_d = nc.dram_tensor("gate_s", [128, ot_g], f32, kind="ExternalInput").ap()
    us_d = nc.dram_tensor("up_s", [128, ot_g], f32, kind="ExternalInput").ap()
    out_d = nc.dram_tensor(
        "out", [ot_d * 128, n_mega * t_mega], f32, kind="ExternalOutput"
    ).ap()

    with tile.TileContext(nc) as tc:
        with (
            tc.tile_pool(name="px", bufs=1) as px,
            tc.tile_pool(name="pw", bufs=2) as pw,
            tc.tile_pool(name="pdw", bufs=2) as pdw,
            tc.tile_pool(name="ph", bufs=1) as ph,
            tc.tile_pool(name="pe", bufs=2) as pe,
            tc.tile_pool(name="po", bufs=3) as po,
            tc.tile_pool(name="pscale", bufs=1) as pscale,
            tc.tile_pool(name="pp", bufs=8, space="PSUM") as pp,
        ):
            gs_t = pscale.tile([128, ot_g], f32, name="gs_t")
            nc.sync.dma_start(out=gs_t[:], in_=gs_d[:])
            us_t = pscale.tile([128, ot_g], f32, name="us_t")
            nc.sync.dma_start(out=us_t[:], in_=us_d[:])

            for m in range(n_mega):
                xh = px.tile([128, ko_g, t_mega], act_dt, tag="xh", name="xh")
                nc.sync.dma_start(out=xh[:], in_=xh_d[m])
                if split:
                    xl = px.tile([128, ko_g, t_mega], act_dt, tag="xl", name="xl")
                    nc.sync.dma_start(out=xl[:], in_=xl_d[m])
                hh = ph.tile([128, ko_d, t_mega], act_dt, tag="hh", name="hh")
                if split:
                    hl = ph.tile([128, ko_d, t_mega], act_dt, tag="hl", name="hl")

                # ---- gate/up matmuls + SwiGLU epilogue ----
                for ot in range(ot_g):
                    gw = pw.tile([128, ko_g, 128], act_dt, tag="gw", name="gw")
                    nc.sync.dma_start(out=gw[:], in_=gw_d[ot])
                    uw = pw.tile([128, ko_g, 128], act_dt, tag="uw", name="uw")
                    nc.sync.dma_start(out=uw[:], in_=uw_d[ot])

                    psg = pp.tile([128, t_mega], f32, tag="ps", name="psg")
                    for k in range(ko_g):
                        nc.tensor.matmul(
                            psg[:], gw[:, k, :], xh[:, k, :],
                            start=(k == 0), stop=(not split and k == ko_g - 1),
                        )
                        if split:
                            nc.tensor.matmul(
                                psg[:], gw[:, k, :], xl[:, k, :],
                                start=False, stop=(k == ko_g - 1),
                            )
                    psu = pp.tile([128, t_mega], f32, tag="ps", name="psu")
                    for k in range(ko_g):
                        nc.tensor.matmul(
                            psu[:], uw[:, k, :], xh[:, k, :],
                            start=(k == 0), stop=(not split and k == ko_g - 1),
                        )
                        if split:
                            nc.tensor.matmul(
                                psu[:], uw[:, k, :], xl[:, k, :],
                                start=False, stop=(k == ko_g - 1),
                            )

                    gact = pe.tile([128, t_mega], f32, tag="gact", name="gact")
                    nc.scalar.activation(
                        gact[:], psg[:], silu, scale=gs_t[:, ot : ot + 1]
                    )
                    # h = (up_psum * up_scale) * silu(gate * gate_scale)
                    if split:
                        prod = pe.tile([128, t_mega], f32, tag="prod", name="prod")
                        nc.vector.scalar_tensor_tensor(
                            prod[:], psu[:], us_t[:, ot : ot + 1], gact[:], mult, mult
                        )
                        nc.vector.tensor_copy(out=hh[:, ot, :], in_=prod[:])
                        nc.vector.tensor_sub(hl[:, ot, :], prod[:], hh[:, ot, :])
                    else:
                        nc.vector.scalar_tensor_tensor(
                            hh[:, ot, :], psu[:], us_t[:, ot : ot + 1], gact[:],
                            mult, mult,
                        )

                # ---- down matmuls (partial sums; scaled on host) ----
                for o2 in range(ot_d):
                    dw = pdw.tile([128, ko_d, 128], act_dt, tag="dw", name="dw")
                    nc.sync.dma_start(out=dw[:], in_=dw_d[o2])
                    pso = pp.tile([128, t_mega], f32, tag="ps", name="pso")
                    for k in range(ko_d):
                        nc.tensor.matmul(
                            pso[:], dw[:, k, :], hh[:, k, :],
                            start=(k == 0), stop=(not split and k == ko_d - 1),
                        )
                        if split:
                            nc.tensor.matmul(
                                pso[:], dw[:, k, :], hl[:, k, :],
                                start=False, stop=(k == ko_d - 1),
                            )
                    ob = po.tile([128, t_mega], f32, tag="ob", name="ob")
                    nc.scalar.copy(ob[:], pso[:])
                    nc.sync.dma_start(
                        out=out_d[
                            o2 * 128 : (o2 + 1) * 128,
                            m * t_mega : (m + 1) * t_mega,
                        ],
                        in_=ob[:],
                    )

    nc.compile()
    return nc


def _get_module(mode, t_mega, n_mega, ko_g, ot_g, ot_d):
    key = (mode, t_mega, n_mega, ko_g, ot_g, ot_d)
    if key not in _nc_cache:
        _nc_cache[key] = _build_module(mode, t_mega, n_mega, ko_g, ot_g, ot_d)
    return _nc_cache[key]


def _prep_x(x, t_mega, n_mega, ko_g, mode):
    """[T, H] f32 -> tiled [mega, ki, ko, t] activations (hi, lo-or-None)."""
    t_total = n_mega * t_mega
    xf = np.ascontiguousarray(x.reshape(t_total, ko_g * 128), dtype=np.float32)
    xr = xf.reshape(n_mega, t_mega, ko_g, 128).transpose(0, 3, 2, 1)
    if mode == "fp16":
        return np.ascontiguousarray(xr).astype(np.float16), None
    x_hi = np.ascontiguousarray(xr).astype(_BF16)
    x_lo = (xr - x_hi.astype(np.float32)).astype(_BF16)
    return x_hi, x_lo


def _prep_w(w_int, ot, ko, mode):
    """[ot*128 (o), ko*128 (k)] int-valued -> [ot, ki, ko, o] contiguous."""
    dt = np.float16 if mode == "fp16" else _BF16
    return np.ascontiguousarray(
        w_int.astype(dt).reshape(ot, 128, ko, 128).transpose(0, 3, 2, 1)
    )


def _prep_scale(s, ot):
    return np.ascontiguousarray(s.reshape(ot, 128).T, dtype=np.float32)


def _run_spmd(nc, in_maps, trace):
    from concourse.bass_utils import run_bass_kernel_spmd

    return run_bass_kernel_spmd(
        nc, in_maps, core_ids=list(range(len(in_maps))), trace=trace
    )


def kernel(x, gate_wq, gate_scale, up_wq, up_scale, down_wq, down_scale):
    mode = MODE
    n_mega = T // T_MEGA
    ko_g = H // 128
    ot_g = I_LOC // 128
    ot_d = H // 128

    nc = _get_module(mode, T_MEGA, n_mega, ko_g, ot_g, ot_d)

    x_hi, x_lo = _prep_x(np.asarray(x), T_MEGA, n_mega, ko_g, mode)
    gate_wq = np.asarray(gate_wq)
    up_wq = np.asarray(up_wq)
    down_wq = np.asarray(down_wq)
    gate_scale = np.asarray(gate_scale, dtype=np.float32)
    up_scale = np.asarray(up_scale, dtype=np.float32)
    down_scale = np.asarray(down_scale, dtype=np.float32)

    in_maps = []
    for c in range(NCORES):
        sl = slice(c * I_LOC, (c + 1) * I_LOC)
        im = {
            "x_hi": x_hi,
            "gate_w": _prep_w(gate_wq[sl], ot_g, ko_g, mode),
            "up_w": _prep_w(up_wq[sl], ot_g, ko_g, mode),
            "down_w": _prep_w(down_wq[:, sl], ot_d, ot_g, mode),
            "gate_s": _prep_scale(gate_scale[sl], ot_g),
            "up_s": _prep_scale(up_scale[sl], ot_g),
        }
        if x_lo is not None:
            im["x_lo"] = x_lo
        in_maps.append(im)

    trace = bool(int(os.environ.get("TRNMLP_TRACE", "0")))
    res = _run_spmd(nc, in_maps, trace)
    if trace:
        kernel.last_results = res

    acc = res.results[0]["out"].astype(np.float32, copy=True)
    for r in res.results[1:]:
        acc += r["out"]
    acc *= down_scale[:, None]
    return np.ascontiguousarray(acc.T).reshape(B, S, H).astype(np.float32)


kernel.last_results = None


# revision 3
# speedup vs baseline: 2.0155x; 1.1258x over previous
"""Trainium2 Bass kernel: Mistral quantized MLP (SwiGLU with int8-valued int32
weights, per-output-channel scales).

  gate = (x @ dequant(gate_wq).T), up = (x @ dequant(up_wq).T)
  h = silu(gate) * up
  out = h @ dequant(down_wq).T

Strategy (8 NeuronCores, tensor-parallel on the intermediate dim I):
  - Core c owns rows [c*I/8, (c+1)*I/8) of gate/up and the matching columns of
    down. Each core computes a full [H, T] partial of the down projection;
    the host sums the 8 partials (the "all-reduce"), applies down_scale, and
    transposes back to [B, S, H].
  - Weights are int8-valued, hence EXACTLY representable in fp16/bf16.
  - Two precision modes (TRNMLP_MODE env var):
      "fp16"       (default): activations in fp16 (11-bit mantissa), weights
                   exact fp16. One matmul pass. ~3.6e-4 L2 rel err.
      "split_bf16": activations split hi/lo into two bf16 tensors, weights
                   exact bf16. Two matmul passes. ~4.3e-6 L2 rel err, 2x PE.
  - Device layout keeps features on partitions, tokens on the free dim:
    x is pre-transposed/tiled on the host to [mega, ki, ko, t]; weights to
    [o_tile, ki, ko, o] so each DMA is contiguous and each matmul is
    lhsT=[128 k, 128 o] stationary x rhs=[128 k, 512 t] moving, fp32 PSUM.
"""

import os

import ml_dtypes
import numpy as np

_BF16 = ml_dtypes.bfloat16

# Problem dims (hardcoded per the task contract).
B, S, H, I = 2, 2048, 4096, 14336
NCORES = 8
I_LOC = I // NCORES  # 1792
T = B * S  # 4096
T_MEGA = 512  # tokens per resident x block (and per-matmul free dim)

MODE = os.environ.get("TRNMLP_MODE", "fp16")  # "fp16" | "split_bf16"

_nc_cache = {}


def _build_module(mode, t_mega, n_mega, ko_g, ot_g, ot_d):
    """Build + compile the (SPMD, identical on all cores) Bass module.

    ko_g: contraction tiles for gate/up (H/128)
    ot_g: output tiles per core for gate/up (I_loc/128); also the down
          contraction tile count
    ot_d: output tiles for down (H/128)
    """
    import concourse.tile as tile
    from concourse import bacc, mybir

    f32 = mybir.dt.float32
    act_dt = mybir.dt.float16 if mode == "fp16" else mybir.dt.bfloat16
    silu = mybir.ActivationFunctionType.Silu
    mult = mybir.AluOpType.mult
    ko_d = ot_g
    split = mode == "split_bf16"

    nc = bacc.Bacc(
        "TRN2",
        target_bir_lowering=False,
        debug=False,
        enable_asserts=False,
        num_devices=NCORES,
    )

    xh_d = nc.dram_tensor(
        "x_hi", [n_mega, 128, ko_g, t_mega], act_dt, kind="ExternalInput"
    ).ap()
    if split:
        xl_d = nc.dram_tensor(
            "x_lo", [n_mega, 128, ko_g, t_mega], act_dt, kind="ExternalInput"
        ).ap()
    gw_d = nc.dram_tensor(
        "gate_w", [ot_g, 128, ko_g, 128], act_dt, kind="ExternalInput"
    ).ap()
    uw_d = nc.dram_tensor(
        "up_w", [ot_g, 128, ko_g, 128], act_dt, kind="ExternalInput"
    ).ap()
    dw_d = nc.dram_tensor(
        "down_w", [ot_d, 128, ko_d, 128], act_dt, kind="ExternalInput"
    ).ap()
    gs_d = nc.dram_tensor("gate_s", [128, ot_g], f32, kind="ExternalInput").ap()
    us_d = nc.dram_tensor("up_s", [128, ot_g], f32, kind="ExternalInput").ap()
    out_d = nc.dram_tensor(
        "out", [ot_d * 128, n_mega * t_mega], f32, kind="ExternalOutput"
    ).ap()

    with tile.TileContext(nc) as tc:
        with (
            tc.tile_pool(name="px", bufs=2) as px,
            tc.tile_pool(name="pw", bufs=2) as pw,
            tc.tile_pool(name="pdw", bufs=4) as pdw,
            tc.tile_pool(name="ph", bufs=2) as ph,
            tc.tile_pool(name="pe", bufs=2) as pe,
            tc.tile_pool(name="po", bufs=3) as po,
            tc.tile_pool(name="pscale", bufs=1) as pscale,
            tc.tile_pool(name="pp", bufs=8, space="PSUM") as pp,
        ):
            gs_t = pscale.tile([128, ot_g], f32, name="gs_t")
            nc.sync.dma_start(out=gs_t[:], in_=gs_d[:])
            us_t = pscale.tile([128, ot_g], f32, name="us_t")
            nc.sync.dma_start(out=us_t[:], in_=us_d[:])

            def g_group(m, ot, xh, xl, hh, hl):
                """Gate+up matmul group for (mega m, out tile ot) + SwiGLU."""
                gw = pw.tile([128, ko_g, 128], act_dt, tag="gw", name="gw")
                nc.sync.dma_start(out=gw[:], in_=gw_d[ot])
                uw = pw.tile([128, ko_g, 128], act_dt, tag="uw", name="uw")
                nc.sync.dma_start(out=uw[:], in_=uw_d[ot])

                psg = pp.tile([128, t_mega], f32, tag="ps", name="psg")
                for k in range(ko_g):
                    nc.tensor.matmul(
                        psg[:], gw[:, k, :], xh[:, k, :],
                        start=(k == 0), stop=(not split and k == ko_g - 1),
                    )
                    if split:
                        nc.tensor.matmul(
                            psg[:], gw[:, k, :], xl[:, k, :],
                            start=False, stop=(k == ko_g - 1),
                        )
                psu = pp.tile([128, t_mega], f32, tag="ps", name="psu")
                for k in range(ko_g):
                    nc.tensor.matmul(
                        psu[:], uw[:, k, :], xh[:, k, :],
                        start=(k == 0), stop=(not split and k == ko_g - 1),
                    )
                    if split:
                        nc.tensor.matmul(
                            psu[:], uw[:, k, :], xl[:, k, :],
                            start=False, stop=(k == ko_g - 1),
                        )

                gact = pe.tile([128, t_mega], f32, tag="gact", name="gact")
                nc.scalar.activation(
                    gact[:], psg[:], silu, scale=gs_t[:, ot : ot + 1]
                )
                # h = (up_psum * up_scale) * silu(gate * gate_scale)
                if split:
                    prod = pe.tile([128, t_mega], f32, tag="prod", name="prod")
                    nc.vector.scalar_tensor_tensor(
                        prod[:], psu[:], us_t[:, ot : ot + 1], gact[:], mult, mult
                    )
                    nc.vector.tensor_copy(out=hh[:, ot, :], in_=prod[:])
                    nc.vector.tensor_sub(hl[:, ot, :], prod[:], hh[:, ot, :])
                else:
                    nc.vector.scalar_tensor_tensor(
                        hh[:, ot, :], psu[:], us_t[:, ot : ot + 1], gact[:],
                        mult, mult,
                    )

            def d_group(m, o2, hh, hl):
                """Down matmul group for (mega m, out tile o2); host scales."""
                dw = pdw.tile([128, ko_d, 128], act_dt, tag="dw", name="dw")
                nc.sync.dma_start(out=dw[:], in_=dw_d[o2])
                pso = pp.tile([128, t_mega], f32, tag="ps", name="pso")
                for k in range(ko_d):
                    nc.tensor.matmul(
                        pso[:], dw[:, k, :], hh[:, k, :],
                        start=(k == 0), stop=(not split and k == ko_d - 1),
                    )
                    if split:
                        nc.tensor.matmul(
                            pso[:], dw[:, k, :], hl[:, k, :],
                            start=False, stop=(k == ko_d - 1),
                        )
                ob = po.tile([128, t_mega], f32, tag="ob", name="ob")
                nc.scalar.copy(ob[:], pso[:])
                nc.sync.dma_start(
                    out=out_d[
                        o2 * 128 : (o2 + 1) * 128,
                        m * t_mega : (m + 1) * t_mega,
                    ],
                    in_=ob[:],
                )

            # Software pipeline: interleave mega m's gate/up groups with mega
            # m-1's down groups, spreading the down-phase DMA (down weights +
            # out stores) across the whole mega so HBM never saturates and the
            # PE never stalls.
            prev = None  # (m-1, hh, hl)
            for m in range(n_mega):
                xh = px.tile([128, ko_g, t_mega], act_dt, tag="xh", name="xh")
                nc.sync.dma_start(out=xh[:], in_=xh_d[m])
                xl = None
                if split:
                    xl = px.tile([128, ko_g, t_mega], act_dt, tag="xl", name="xl")
                    nc.sync.dma_start(out=xl[:], in_=xl_d[m])
                hh = ph.tile([128, ko_d, t_mega], act_dt, tag="hh", name="hh")
                hl = None
                if split:
                    hl = ph.tile([128, ko_d, t_mega], act_dt, tag="hl", name="hl")

                for ot in range(ot_g):
                    g_group(m, ot, xh, xl, hh, hl)
                    if prev is not None:
                        pm, phh, phl = prev
                        for o2 in range(
                            ot_d * ot // ot_g, ot_d * (ot + 1) // ot_g
                        ):
                            d_group(pm, o2, phh, phl)
                prev = (m, hh, hl)

            pm, phh, phl = prev
            for o2 in range(ot_d):
                d_group(pm, o2, phh, phl)

    nc.compile()
    return nc


def _get_module(mode, t_mega, n_mega, ko_g, ot_g, ot_d):
    key = (mode, t_mega, n_mega, ko_g, ot_g, ot_d)
    if key not in _nc_cache:
        _nc_cache[key] = _build_module(mode, t_mega, n_mega, ko_g, ot_g, ot_d)
    return _nc_cache[key]


def _prep_x(x, t_mega, n_mega, ko_g, mode):
    """[T, H] f32 -> tiled [mega, ki, ko, t] activations (hi, lo-or-None)."""
    t_total = n_mega * t_mega
    xf = np.ascontiguousarray(x.reshape(t_total, ko_g * 128), dtype=np.float32)
    xr = xf.reshape(n_mega, t_mega, ko_g, 128).transpose(0, 3, 2, 1)
    if mode == "fp16":
        return np.ascontiguousarray(xr).astype(np.float16), None
    x_hi = np.ascontiguousarray(xr).astype(_BF16)
    x_lo = (xr - x_hi.astype(np.float32)).astype(_BF16)
    return x_hi, x_lo


def _prep_w(w_int, ot, ko, mode):
    """[ot*128 (o), ko*128 (k)] int-valued -> [ot, ki, ko, o] contiguous."""
    dt = np.float16 if mode == "fp16" else _BF16
    return np.ascontiguousarray(
        w_int.astype(dt).reshape(ot, 128, ko, 128).transpose(0, 3, 2, 1)
    )


def _prep_scale(s, ot):
    return np.ascontiguousarray(s.reshape(ot, 128).T, dtype=np.float32)


def _run_spmd(nc, in_maps, trace):
    from concourse.bass_utils import run_bass_kernel_spmd

    return run_bass_kernel_spmd(
        nc, in_maps, core_ids=list(range(len(in_maps))), trace=trace
    )


def kernel(x, gate_wq, gate_scale, up_wq, up_scale, down_wq, down_scale):
    mode = MODE
    n_mega = T // T_MEGA
    ko_g = H // 128
    ot_g = I_LOC // 128
    ot_d = H // 128

    nc = _get_module(mode, T_MEGA, n_mega, ko_g, ot_g, ot_d)

    x_hi, x_lo = _prep_x(np.asarray(x), T_MEGA, n_mega, ko_g, mode)
    gate_wq = np.asarray(gate_wq)
    up_wq = np.asarray(up_wq)
    down_wq = np.asarray(down_wq)
    gate_scale = np.asarray(gate_scale, dtype=np.float32)
    up_scale = np.asarray(up_scale, dtype=np.float32)
    down_scale = np.asarray(down_scale, dtype=np.float32)

    in_maps = []
    for c in range(NCORES):
        sl = slice(c * I_LOC, (c + 1) * I_LOC)
        im = {
            "x_hi": x_hi,
            "gate_w": _prep_w(gate_wq[sl], ot_g, ko_g, mode),
            "up_w": _prep_w(up_wq[sl], ot_g, ko_g, mode),
            "down_w": _prep_w(down_wq[:, sl], ot_d, ot_g, mode),
            "gate_s": _prep_scale(gate_scale[sl], ot_g),
            "up_s": _prep_scale(up_scale[sl], ot_g),
        }
        if x_lo is not None:
            im["x_lo"] = x_lo
        in_maps.append(im)

    trace = bool(int(os.environ.get("TRNMLP_TRACE", "0")))
    res = _run_spmd(nc, in_maps, trace)
    if trace:
        kernel.last_results = res

    acc = res.results[0]["out"].astype(np.float32, copy=True)
    for r in res.results[1:]:
        acc += r["out"]
    acc *= down_scale[:, None]
    return np.ascontiguousarray(acc.T).reshape(B, S, H).astype(np.float32)


kernel.last_results = None


# revision 4
# speedup vs baseline: 2.0185x; 1.0015x over previous
"""Trainium2 Bass kernel: Mistral quantized MLP (SwiGLU with int8-valued int32
weights, per-output-channel scales).

  gate = (x @ dequant(gate_wq).T), up = (x @ dequant(up_wq).T)
  h = silu(gate) * up
  out = h @ dequant(down_wq).T

Strategy (8 NeuronCores, tensor-parallel on the intermediate dim I):
  - Core c owns rows [c*I/8, (c+1)*I/8) of gate/up and the matching columns of
    down. Each core computes a full [H, T] partial of the down projection;
    the host sums the 8 partials (the "all-reduce"), applies down_scale, and
    transposes back to [B, S, H].
  - Weights are int8-valued, hence EXACTLY representable in fp16/bf16.
  - Two precision modes (TRNMLP_MODE env var):
      "fp16"       (default): activations in fp16 (11-bit mantissa), weights
                   exact fp16. One matmul pass. ~3.6e-4 L2 rel err.
      "split_bf16": activations split hi/lo into two bf16 tensors, weights
                   exact bf16. Two matmul passes. ~4.3e-6 L2 rel err, 2x PE.
  - Device layout keeps features on partitions, tokens on the free dim:
    x is pre-transposed/tiled on the host to [mega, ki, ko, t]; weights to
    [o_tile, ki, ko, o] so each DMA is contiguous and each matmul is
    lhsT=[128 k, 128 o] stationary x rhs=[128 k, 512 t] moving, fp32 PSUM.
"""

import os

import ml_dtypes
import numpy as np

_BF16 = ml_dtypes.bfloat16

# Problem dims (hardcoded per the task contract).
B, S, H, I = 2, 2048, 4096, 14336
NCORES = 8
I_LOC = I // NCORES  # 1792
T = B * S  # 4096
T_MEGA = 512  # tokens per resident x block (and per-matmul free dim)

MODE = os.environ.get("TRNMLP_MODE", "fp16")  # "fp16" | "split_bf16"

_nc_cache = {}


def _build_module(mode, t_mega, n_mega, ko_g, ot_g, ot_d):
    """Build + compile the (SPMD, identical on all cores) Bass module.

    ko_g: contraction tiles for gate/up (H/128)
    ot_g: output tiles per core for gate/up (I_loc/128); also the down
          contraction tile count
    ot_d: output tiles for down (H/128)
    """
    import concourse.tile as tile
    from concourse import bacc, mybir

    f32 = mybir.dt.float32
    act_dt = mybir.dt.float16 if mode == "fp16" else mybir.dt.bfloat16
    silu = mybir.ActivationFunctionType.Silu
    mult = mybir.AluOpType.mult
    ko_d = ot_g
    split = mode == "split_bf16"

    nc = bacc.Bacc(
        "TRN2",
        target_bir_lowering=False,
        debug=False,
        enable_asserts=False,
        num_devices=NCORES,
    )

    xh_d = nc.dram_tensor(
        "x_hi", [n_mega, 128, ko_g, t_mega], act_dt, kind="ExternalInput"
    ).ap()
    if split:
        xl_d = nc.dram_tensor(
            "x_lo", [n_mega, 128, ko_g, t_mega], act_dt, kind="ExternalInput"
        ).ap()
    gw_d = nc.dram_tensor(
        "gate_w", [ot_g, 128, ko_g, 128], act_dt, kind="ExternalInput"
    ).ap()
    uw_d = nc.dram_tensor(
        "up_w", [ot_g, 128, ko_g, 128], act_dt, kind="ExternalInput"
    ).ap()
    dw_d = nc.dram_tensor(
        "down_w", [ot_d, 128, ko_d, 128], act_dt, kind="ExternalInput"
    ).ap()
    gs_d = nc.dram_tensor("gate_s", [128, ot_g], f32, kind="ExternalInput").ap()
    us_d = nc.dram_tensor("up_s", [128, ot_g], f32, kind="ExternalInput").ap()
    out_d = nc.dram_tensor(
        "out", [ot_d * 128, n_mega * t_mega], f32, kind="ExternalOutput"
    ).ap()

    with tile.TileContext(nc) as tc:
        with (
            tc.tile_pool(name="px", bufs=1 if split else 2) as px,
            tc.tile_pool(name="pw", bufs=2) as pw,
            tc.tile_pool(name="pdw", bufs=2 if split else 4) as pdw,
            tc.tile_pool(name="ph", bufs=2) as ph,
            tc.tile_pool(name="pe", bufs=2) as pe,
            tc.tile_pool(name="po", bufs=3) as po,
            tc.tile_pool(name="pscale", bufs=1) as pscale,
            tc.tile_pool(name="pp", bufs=8, space="PSUM") as pp,
        ):
            gs_t = pscale.tile([128, ot_g], f32, name="gs_t")
            nc.sync.dma_start(out=gs_t[:], in_=gs_d[:])
            us_t = pscale.tile([128, ot_g], f32, name="us_t")
            nc.sync.dma_start(out=us_t[:], in_=us_d[:])

            def g_group(m, ot, xh, xl, hh, hl):
                """Gate+up matmul group for (mega m, out tile ot) + SwiGLU."""
                gw = pw.tile([128, ko_g, 128], act_dt, tag="gw", name="gw")
                nc.sync.dma_start(out=gw[:], in_=gw_d[ot])
                uw = pw.tile([128, ko_g, 128], act_dt, tag="uw", name="uw")
                nc.sync.dma_start(out=uw[:], in_=uw_d[ot])

                psg = pp.tile([128, t_mega], f32, tag="ps", name="psg")
                for k in range(ko_g):
                    nc.tensor.matmul(
                        psg[:], gw[:, k, :], xh[:, k, :],
                        start=(k == 0), stop=(not split and k == ko_g - 1),
                    )
                    if split:
                        nc.tensor.matmul(
                            psg[:], gw[:, k, :], xl[:, k, :],
                            start=False, stop=(k == ko_g - 1),
                        )
                psu = pp.tile([128, t_mega], f32, tag="ps", name="psu")
                for k in range(ko_g):
                    nc.tensor.matmul(
                        psu[:], uw[:, k, :], xh[:, k, :],
                        start=(k == 0), stop=(not split and k == ko_g - 1),
                    )
                    if split:
                        nc.tensor.matmul(
                            psu[:], uw[:, k, :], xl[:, k, :],
                            start=False, stop=(k == ko_g - 1),
                        )

                gact = pe.tile([128, t_mega], f32, tag="gact", name="gact")
                nc.scalar.activation(
                    gact[:], psg[:], silu, scale=gs_t[:, ot : ot + 1]
                )
                # h = (up_psum * up_scale) * silu(gate * gate_scale)
                if split:
                    prod = pe.tile([128, t_mega], f32, tag="prod", name="prod")
                    nc.vector.scalar_tensor_tensor(
                        prod[:], psu[:], us_t[:, ot : ot + 1], gact[:], mult, mult
                    )
                    nc.vector.tensor_copy(out=hh[:, ot, :], in_=prod[:])
                    nc.vector.tensor_sub(hl[:, ot, :], prod[:], hh[:, ot, :])
                else:
                    nc.vector.scalar_tensor_tensor(
                        hh[:, ot, :], psu[:], us_t[:, ot : ot + 1], gact[:],
                        mult, mult,
                    )

            def d_group(m, o2, hh, hl):
                """Down matmul group for (mega m, out tile o2); host scales."""
                dw = pdw.tile([128, ko_d, 128], act_dt, tag="dw", name="dw")
                nc.sync.dma_start(out=dw[:], in_=dw_d[o2])
                pso = pp.tile([128, t_mega], f32, tag="ps", name="pso")
                for k in range(ko_d):
                    nc.tensor.matmul(
                        pso[:], dw[:, k, :], hh[:, k, :],
                        start=(k == 0), stop=(not split and k == ko_d - 1),
                    )
                    if split:
                        nc.tensor.matmul(
                            pso[:], dw[:, k, :], hl[:, k, :],
                            start=False, stop=(k == ko_d - 1),
                        )
                ob = po.tile([128, t_mega], f32, tag="ob", name="ob")
                nc.scalar.copy(ob[:], pso[:])
                nc.sync.dma_start(
                    out=out_d[
                        o2 * 128 : (o2 + 1) * 128,
                        m * t_mega : (m + 1) * t_mega,
                    ],
                    in_=ob[:],
                )

            # Software pipeline: interleave mega m's gate/up groups with mega
            # m-1's down groups, spreading the down-phase DMA (down weights +
            # out stores) across the whole mega so HBM never saturates and the
            # PE never stalls.
            prev = None  # (m-1, hh, hl)
            for m in range(n_mega):
                xh = px.tile([128, ko_g, t_mega], act_dt, tag="xh", name="xh")
                nc.sync.dma_start(out=xh[:], in_=xh_d[m])
                xl = None
                if split:
                    xl = px.tile([128, ko_g, t_mega], act_dt, tag="xl", name="xl")
                    nc.sync.dma_start(out=xl[:], in_=xl_d[m])
                hh = ph.tile([128, ko_d, t_mega], act_dt, tag="hh", name="hh")
                hl = None
                if split:
                    hl = ph.tile([128, ko_d, t_mega], act_dt, tag="hl", name="hl")

                for ot in range(ot_g):
                    g_group(m, ot, xh, xl, hh, hl)
                    if prev is not None:
                        pm, phh, phl = prev
                        for o2 in range(
                            ot_d * ot // ot_g, ot_d * (ot + 1) // ot_g
                        ):
                            d_group(pm, o2, phh, phl)
                prev = (m, hh, hl)

            pm, phh, phl = prev
            for o2 in range(ot_d):
                d_group(pm, o2, phh, phl)

    nc.compile()
    return nc


def _get_module(mode, t_mega, n_mega, ko_g, ot_g, ot_d):
    key = (mode, t_mega, n_mega, ko_g, ot_g, ot_d)
    if key not in _nc_cache:
        _nc_cache[key] = _build_module(mode, t_mega, n_mega, ko_g, ot_g, ot_d)
    return _nc_cache[key]


def _prep_x(x, t_mega, n_mega, ko_g, mode):
    """[T, H] f32 -> tiled [mega, ki, ko, t] activations (hi, lo-or-None)."""
    t_total = n_mega * t_mega
    xf = np.ascontiguousarray(x.reshape(t_total, ko_g * 128), dtype=np.float32)
    xr = xf.reshape(n_mega, t_mega, ko_g, 128).transpose(0, 3, 2, 1)
    if mode == "fp16":
        return np.ascontiguousarray(xr).astype(np.float16), None
    x_hi = np.ascontiguousarray(xr).astype(_BF16)
    x_lo = (xr - x_hi.astype(np.float32)).astype(_BF16)
    return x_hi, x_lo


def _prep_w(w_int, ot, ko, mode):
    """[ot*128 (o), ko*128 (k)] int-valued -> [ot, ki, ko, o] contiguous."""
    dt = np.float16 if mode == "fp16" else _BF16
    return np.ascontiguousarray(
        w_int.astype(dt).reshape(ot, 128, ko, 128).transpose(0, 3, 2, 1)
    )


def _prep_scale(s, ot):
    return np.ascontiguousarray(s.reshape(ot, 128).T, dtype=np.float32)


def _run_spmd(nc, in_maps, trace):
    from concourse.bass_utils import run_bass_kernel_spmd

    return run_bass_kernel_spmd(
        nc, in_maps, core_ids=list(range(len(in_maps))), trace=trace
    )


def kernel(x, gate_wq, gate_scale, up_wq, up_scale, down_wq, down_scale):
    mode = MODE
    n_mega = T // T_MEGA
    ko_g = H // 128
    ot_g = I_LOC // 128
    ot_d = H // 128

    nc = _get_module(mode, T_MEGA, n_mega, ko_g, ot_g, ot_d)

    x_hi, x_lo = _prep_x(np.asarray(x), T_MEGA, n_mega, ko_g, mode)
    gate_wq = np.asarray(gate_wq)
    up_wq = np.asarray(up_wq)
    down_wq = np.asarray(down_wq)
    gate_scale = np.asarray(gate_scale, dtype=np.float32)
    up_scale = np.asarray(up_scale, dtype=np.float32)
    down_scale = np.asarray(down_scale, dtype=np.float32)

    in_maps = []
    for c in range(NCORES):
        sl = slice(c * I_LOC, (c + 1) * I_LOC)
        im = {
            "x_hi": x_hi,
            "gate_w": _prep_w(gate_wq[sl], ot_g, ko_g, mode),
            "up_w": _prep_w(up_wq[sl], ot_g, ko_g, mode),
            "down_w": _prep_w(down_wq[:, sl], ot_d, ot_g, mode),
            "gate_s": _prep_scale(gate_scale[sl], ot_g),
            "up_s": _prep_scale(up_scale[sl], ot_g),
        }
        if x_lo is not None:
            im["x_lo"] = x_lo
        in_maps.append(im)

    trace = bool(int(os.environ.get("TRNMLP_TRACE", "0")))
    res = _run_spmd(nc, in_maps, trace)
    if trace:
        kernel.last_results = res

    acc = res.results[0]["out"].astype(np.float32, copy=True)
    for r in res.results[1:]:
        acc += r["out"]
    acc *= down_scale[:, None]
    return np.ascontiguousarray(acc.T).reshape(B, S, H).astype(np.float32)


kernel.last_results = None
